# revision 19
# baseline (speedup 1.0000x reference)
# Trainium2 Bass kernel for a binarized 2-block MLP (BNN):
#   h1 = sign(BN1(x @ sign(w1).T + b1)); h2 = sign(BN2(h1 @ sign(w2).T + b2))
#   out = log_softmax(h2 @ sign(w5).T + b5)
#
# Strategy: pure data parallel over 8 NeuronCores (batch sharded, weights
# replicated). Host-side prep:
#   * x is split into fp16 hi/lo parts (x == xh + xl/2048 exactly in fp32,
#     since the lo residual has <= 13 significant bits). Both stream through
#     the PE at full (1 col/cycle) rate vs fp32's 4 passes; with +-1 binary
#     weights every product is exact in the PE's FP22 pipe, so the result
#     matches a true fp32 matmul to accumulation order.
#   * x shards are pre-swizzled to [128, chunk, kslice, col] so the
#     contraction dim lands on SBUF partitions and every partition's
#     per-chunk DMA slice is one contiguous 16KB run (HBM line rate).
#   * BN is folded into per-feature scale/shift applied inside the Sign
#     activation (ACT computes func(scale*in + bias) for free).
#   * b5 is folded in on the vector engine: sum(exp(mm+b5)) via a fused
#     multiply-reduce against exp(b5), and the final subtract adds b5 in the
#     same fused op. Keeps the PE free of bias matmuls.
# fc5 output lands batch-on-partitions (y2 block as the stationary operand)
# so log_softmax reduces along the free dim; the result is PE-transposed so
# the store DMA has 512B-contiguous runs; host reassembles.

import os
import sys

import numpy as np

for _p in ("/opt/trn_rl_repo", "/root/.axon_site/_ro/trn_rl_repo"):
    if os.path.isdir(_p) and _p not in sys.path:
        sys.path.insert(0, _p)

import concourse.bass as bass
import concourse.mybir as mybir
import concourse.tile as tile
from concourse import bacc
from concourse.masks import make_identity

N_CORES = 8
B, D, H1, H2, O = 65536, 1024, 50, 20, 10
BPC = B // N_CORES  # batch rows per core
CH = 512            # batch chunk (one PSUM bank of fp32)
KS = D // 128       # contraction slices
EPS = 1e-4
LO = 2048.0         # lo-part scale (2**11)

F16 = mybir.dt.float16
F32 = mybir.dt.float32
AF = mybir.ActivationFunctionType
AX = mybir.AxisListType
OP = mybir.AluOpType


def build_bass(bpc: int = BPC) -> bass.Bass:
    nch = bpc // CH
    npair = nch // 2
    nc = bacc.Bacc("TRN2", target_bir_lowering=False)

    # All activations used here (Sign, Exp, Ln) live together in the
    # "natural_log_exp_and_others" ACT table set, but the default chooser
    # first-matches Exp->exp_and_others and Ln->natural_log, reloading
    # tables twice per chunk (~2.7us each). Restrict the chooser to the
    # combined set (other entries emptied so indices stay aligned with
    # act_info.json) -> exactly one table load for the whole kernel.
    def _act_table_loads_combined_set_only(self=nc):
        import bass_rust as _br

        from concourse.hw_specs import get_activation_tables

        has_act = any(
            isinstance(i, mybir.InstActivation)
            for blk in self.main_func.blocks
            for i in blk.instructions
        )
        if not has_act:
            return
        tabs = get_activation_tables(self.m.arch)
        tables = [
            (name, fns if name == "natural_log_exp_and_others" else set())
            for name, fns in tabs.items()
        ]
        _br.insert_act_table_loads(self, tables)

    nc.insert_act_table_loads = _act_table_loads_combined_set_only

    # x parts arrive pre-swizzled: xs[p, b, k, n] = x.T[k*128+p, b*CH+n],
    # so each partition's per-pair (2-chunk) DMA slice is contiguous 16KB.
    xh = nc.declare_dram_parameter("xh", [128, nch, KS, CH], F16, isOutput=False)
    xl = nc.declare_dram_parameter("xl", [128, nch, KS, CH], F16, isOutput=False)
    w1h = nc.declare_dram_parameter("w1h", [D, H1], F16, isOutput=False)
    w1l = nc.declare_dram_parameter("w1l", [D, H1], F16, isOutput=False)
    w2t = nc.declare_dram_parameter("w2t", [H1, H2], F16, isOutput=False)
    w5t = nc.declare_dram_parameter("w5t", [H2, O], F16, isOutput=False)
    cs1 = nc.declare_dram_parameter("cs1", [H1, 2], F32, isOutput=False)
    cs2 = nc.declare_dram_parameter("cs2", [H2, 2], F32, isOutput=False)
    b5r = nc.declare_dram_parameter("b5r", [128, 4 * O], F32, isOutput=False)
    eb5r = nc.declare_dram_parameter("eb5r", [128, 4 * O], F32, isOutput=False)
    # Output, transposed blocks: y[j*O + o, b*128 + p] = out[b*512 + j*128 + p, o]
    y = nc.declare_dram_parameter("y", [4 * O, nch * 128], F32, isOutput=True)

    with tile.TileContext(nc) as tc:
        from contextlib import ExitStack

        with ExitStack() as ctx:
            singles = ctx.enter_context(tc.tile_pool(name="singles", bufs=1))
            xpool = ctx.enter_context(tc.tile_pool(name="xpool", bufs=4))
            mids = ctx.enter_context(tc.tile_pool(name="mids", bufs=3))
            outs = ctx.enter_context(tc.tile_pool(name="outs", bufs=3))
            p1pool = ctx.enter_context(tc.tile_pool(name="p1", bufs=2, space="PSUM"))
            p2pool = ctx.enter_context(tc.tile_pool(name="p2", bufs=1, space="PSUM"))
            p5pool = ctx.enter_context(tc.tile_pool(name="p5", bufs=2, space="PSUM"))
            pTpool = ctx.enter_context(tc.tile_pool(name="pT", bufs=1, space="PSUM"))

            # --- constants / weights (loaded once) ---
            w1h_sb = singles.tile([128, KS, H1], F16)
            nc.sync.dma_start(out=w1h_sb, in_=w1h.rearrange("(k p) m -> p k m", p=128))
            w1l_sb = singles.tile([128, KS, H1], F16)
            nc.sync.dma_start(out=w1l_sb, in_=w1l.rearrange("(k p) m -> p k m", p=128))
            w2_sb = singles.tile([H1, H2], F16)
            nc.sync.dma_start(out=w2_sb, in_=w2t[:, :])
            w5_sb = singles.tile([H2, O], F16)
            nc.sync.dma_start(out=w5_sb, in_=w5t[:, :])
            cs1_sb = singles.tile([H1, 2], F32)
            nc.sync.dma_start(out=cs1_sb, in_=cs1[:, :])
            cs2_sb = singles.tile([H2, 2], F32)
            nc.sync.dma_start(out=cs2_sb, in_=cs2[:, :])
            b5r_sb = singles.tile([128, 4 * O], F32)
            nc.sync.dma_start(out=b5r_sb, in_=b5r[:, :])
            eb5r_sb = singles.tile([128, 4 * O], F32)
            nc.sync.dma_start(out=eb5r_sb, in_=eb5r[:, :])
            ident = singles.tile([128, 128], F32)
            make_identity(nc, ident)

            pending = []  # deferred (out_t, chunk) transpose+store

            def store(out_t, b):
                psT = pTpool.tile([4 * O, 128], F32, tag="psT", name="psT")
                nc.tensor.transpose(psT, out_t, ident)
                oT = outs.tile([4 * O, 128], F32, tag="oT", name="oT")
                nc.vector.tensor_copy(oT, psT)
                # SWDGE store: keeps SP's HWDGE FIFO free for input DMAs
                nc.gpsimd.dma_start(out=y[:, b * 128:(b + 1) * 128], in_=oT)

            def epilogue(ps1, b):
                """sign1 -> fc2 -> sign2 -> fc5 -> log_softmax for one chunk."""
                y1 = mids.tile([H1, CH], F16, tag="y1", name="y1")
                nc.scalar.activation(y1, ps1, AF.Sign,
                                     bias=cs1_sb[:, 1:2], scale=cs1_sb[:, 0:1])
                ps2 = p2pool.tile([H2, CH], F32, tag="ps2", name="ps2")
                nc.tensor.matmul(ps2, lhsT=w2_sb, rhs=y1, start=True, stop=True)
                y2 = mids.tile([H2, CH], F16, tag="y2", name="y2")
                nc.scalar.activation(y2, ps2, AF.Sign,
                                     bias=cs2_sb[:, 1:2], scale=cs2_sb[:, 0:1])

                ps5 = p5pool.tile([128, 4, O], F32, tag="ps5", name="ps5")
                for j in range(4):
                    nc.tensor.matmul(ps5[:, j, :], lhsT=y2[:, j * 128:(j + 1) * 128],
                                     rhs=w5_sb, start=True, stop=True)

                # log_softmax along free dim; b5 folded in via exp(b5) weights
                # (|logits| <= 21 so no max-subtraction is needed)
                e = mids.tile([128, 4, O], F32, tag="e", name="e")
                nc.scalar.activation(e, ps5, AF.Exp)
                e2 = mids.tile([128, 4, O], F32, tag="e2", name="e2")
                nc.vector.tensor_tensor(
                    out=e2, in0=e, in1=eb5r_sb.rearrange("p (j o) -> p j o", o=O),
                    op=OP.mult)
                s = mids.tile([128, 4], F32, tag="s", name="s")
                nc.vector.reduce_sum(s, e2, axis=AX.X)
                lse = mids.tile([128, 4], F32, tag="lse", name="lse")
                nc.scalar.activation(lse, s, AF.Ln)
                out_t = outs.tile([128, 4 * O], F32, tag="out", name="out_t")
                for j in range(4):
                    js = slice(j * O, (j + 1) * O)
                    nc.vector.scalar_tensor_tensor(
                        out=out_t[:, js], in0=ps5[:, j, :], scalar=lse[:, j:j + 1],
                        in1=b5r_sb[:, js], op0=OP.subtract, op1=OP.add)
                pending.append((out_t, b))

            # Software pipeline: pair i's fc1 matmul stream issues first; the
            # previous pair's epilogue (whose ACT/DVE stages then overlap this
            # fc1 stream) and older stores follow, so the PE never idles on
            # the ACT round-trips and the DMA slots recycle at full rate.
            prev = None
            for pr in range(npair):
                xh_t = xpool.tile([128, 2, KS, CH], F16, tag="xh", name="xh_t")
                nc.sync.dma_start(out=xh_t, in_=xh[:, 2 * pr:2 * pr + 2, :, :])
                xl_t = xpool.tile([128, 2, KS, CH], F16, tag="xl", name="xl_t")
                nc.sync.dma_start(out=xl_t, in_=xl[:, 2 * pr:2 * pr + 2, :, :])

                # fc1 for both chunks of the pair, sharing each LDWEIGHTS
                ps1a = p1pool.tile([H1, CH], F32, tag="ps1a", name="ps1a")
                ps1b = p1pool.tile([H1, CH], F32, tag="ps1b", name="ps1b")
                for k in range(KS):
                    nc.tensor.matmul(ps1a, lhsT=w1h_sb[:, k, :], rhs=xh_t[:, 0, k, :],
                                     start=(k == 0), stop=False)
                    nc.tensor.matmul(ps1b, lhsT=w1h_sb[:, k, :], rhs=xh_t[:, 1, k, :],
                                     start=(k == 0), stop=False)
                    nc.tensor.matmul(ps1a, lhsT=w1l_sb[:, k, :], rhs=xl_t[:, 0, k, :],
                                     start=False, stop=(k == KS - 1))
                    nc.tensor.matmul(ps1b, lhsT=w1l_sb[:, k, :], rhs=xl_t[:, 1, k, :],
                                     start=False, stop=(k == KS - 1))

                if prev is not None:
                    epilogue(prev[0], prev[2])
                    epilogue(prev[1], prev[2] + 1)
                while len(pending) > 2:
                    store(*pending.pop(0))
                prev = (ps1a, ps1b, 2 * pr)

            epilogue(prev[0], prev[2])
            epilogue(prev[1], prev[2] + 1)
            for p_ in pending:
                store(*p_)

    nc.finalize()
    return nc


def _prep_inputs(x, w1, b1, g1, be1, m1, v1, w2, b2, g2, be2, m2, v2, w5, b5,
                 bpc: int = BPC, n_cores: int = N_CORES):
    f64 = np.float64
    w1s = np.where(w1 >= 0, 1.0, -1.0).astype(np.float32)
    w2s = np.where(w2 >= 0, 1.0, -1.0).astype(np.float32)
    w5s = np.where(w5 >= 0, 1.0, -1.0).astype(np.float32)

    w1h = np.ascontiguousarray(w1s.T).astype(np.float16)          # [D, H1]
    w1l = (np.ascontiguousarray(w1s.T) / LO).astype(np.float16)   # +-2**-11
    w2t = np.ascontiguousarray(w2s.T).astype(np.float16)          # [H1, H2]
    w5t = np.ascontiguousarray(w5s.T).astype(np.float16)          # [H2, O]

    b5f = b5.astype(np.float32)
    b5r = np.broadcast_to(np.tile(b5f, 4)[None, :], (128, 4 * O)).copy()
    eb5 = np.exp(b5.astype(f64)).astype(np.float32)
    eb5r = np.broadcast_to(np.tile(eb5, 4)[None, :], (128, 4 * O)).copy()

    s1 = (g1.astype(f64) / np.sqrt(v1.astype(f64) + EPS))
    t1 = s1 * (b1.astype(f64) - m1.astype(f64)) + be1.astype(f64)
    cs1 = np.stack([s1, t1], axis=1).astype(np.float32)           # [H1, 2]
    s2 = (g2.astype(f64) / np.sqrt(v2.astype(f64) + EPS))
    t2 = s2 * (b2.astype(f64) - m2.astype(f64)) + be2.astype(f64)
    cs2 = np.stack([s2, t2], axis=1).astype(np.float32)           # [H2, 2]

    x = np.asarray(x, dtype=np.float32)
    xh = x.astype(np.float16)
    xl = ((x - xh.astype(np.float32)) * LO).astype(np.float16)

    def swizzle(a):  # [bpc, D] -> [128, nch, KS, CH] (see build_bass)
        nch = bpc // CH
        return np.ascontiguousarray(
            a.T.reshape(KS, 128, nch, CH).transpose(1, 2, 0, 3))

    in_maps = []
    for c in range(n_cores):
        rs = slice(c * bpc, (c + 1) * bpc)
        in_maps.append({
            "xh": swizzle(xh[rs]),
            "xl": swizzle(xl[rs]),
            "w1h": w1h, "w1l": w1l, "w2t": w2t, "w5t": w5t,
            "cs1": cs1, "cs2": cs2, "b5r": b5r, "eb5r": eb5r,
        })
    return in_maps


def _decode_output(y_dev: np.ndarray, bpc: int) -> np.ndarray:
    # y_dev [4*O, (bpc/512)*128]: y_dev[j*O+o, b*128+p] = out[b*512+j*128+p, o]
    nch = bpc // CH
    return np.ascontiguousarray(
        y_dev.reshape(4, O, nch, 128).transpose(2, 0, 3, 1).reshape(bpc, O))


_CACHED = {}


def kernel(**inputs) -> np.ndarray:
    from concourse.bass_utils import run_bass_kernel_spmd

    in_maps = _prep_inputs(**inputs)
    if "nc" not in _CACHED:
        _CACHED["nc"] = build_bass()
    nc = _CACHED["nc"]
    res = run_bass_kernel_spmd(nc, in_maps, list(range(N_CORES)))
    out = np.empty((B, O), dtype=np.float32)
    for c in range(N_CORES):
        out[c * BPC:(c + 1) * BPC] = _decode_output(res.results[c]["y"], BPC)
    return out


# revision 22
# speedup vs baseline: 1.0479x; 1.0479x over previous
# Trainium2 Bass kernel for a binarized 2-block MLP (BNN):
#   h1 = sign(BN1(x @ sign(w1).T + b1)); h2 = sign(BN2(h1 @ sign(w2).T + b2))
#   out = log_softmax(h2 @ sign(w5).T + b5)
#
# Strategy: pure data parallel over 8 NeuronCores (batch sharded, weights
# replicated). Host-side prep:
#   * x is split into fp16 hi/lo parts (x == xh + xl/2048 exactly in fp32,
#     since the lo residual has <= 13 significant bits). Both stream through
#     the PE at full (1 col/cycle) rate vs fp32's 4 passes; with +-1 binary
#     weights every product is exact in the PE's FP22 pipe, so the result
#     matches a true fp32 matmul to accumulation order.
#   * x shards are pre-swizzled to [128, chunk, kslice, col] so the
#     contraction dim lands on SBUF partitions and every partition's
#     per-chunk DMA slice is one contiguous 16KB run (HBM line rate).
#   * BN is folded into per-feature scale/shift applied inside the Sign
#     activation (ACT computes func(scale*in + bias) for free).
#   * b5 is folded in on the vector engine: sum(exp(mm+b5)) via a fused
#     multiply-reduce against exp(b5), and the final subtract adds b5 in the
#     same fused op. Keeps the PE free of bias matmuls.
# fc5 output lands batch-on-partitions (y2 block as the stationary operand)
# so log_softmax reduces along the free dim; the result is PE-transposed so
# the store DMA has 512B-contiguous runs; host reassembles.

import os
import sys

import numpy as np

for _p in ("/opt/trn_rl_repo", "/root/.axon_site/_ro/trn_rl_repo"):
    if os.path.isdir(_p) and _p not in sys.path:
        sys.path.insert(0, _p)

import concourse.bass as bass
import concourse.mybir as mybir
import concourse.tile as tile
from concourse import bacc
from concourse.masks import make_identity

N_CORES = 8
B, D, H1, H2, O = 65536, 1024, 50, 20, 10
BPC = B // N_CORES  # batch rows per core
CH = 512            # batch chunk (one PSUM bank of fp32)
KS = D // 128       # contraction slices
EPS = 1e-4
LO = 2048.0         # lo-part scale (2**11)

F16 = mybir.dt.float16
F32 = mybir.dt.float32
AF = mybir.ActivationFunctionType
AX = mybir.AxisListType
OP = mybir.AluOpType


def build_bass(bpc: int = BPC) -> bass.Bass:
    nch = bpc // CH
    npair = nch // 2
    nc = bacc.Bacc("TRN2", target_bir_lowering=False)

    # All activations used here (Sign, Exp, Ln) live together in the
    # "natural_log_exp_and_others" ACT table set, but the default chooser
    # first-matches Exp->exp_and_others and Ln->natural_log, reloading
    # tables twice per chunk (~2.7us each). Restrict the chooser to the
    # combined set (other entries emptied so indices stay aligned with
    # act_info.json) -> exactly one table load for the whole kernel.
    def _act_table_loads_combined_set_only(self=nc):
        import bass_rust as _br

        from concourse.hw_specs import get_activation_tables

        has_act = any(
            isinstance(i, mybir.InstActivation)
            for blk in self.main_func.blocks
            for i in blk.instructions
        )
        if not has_act:
            return
        tabs = get_activation_tables(self.m.arch)
        tables = [
            (name, fns if name == "natural_log_exp_and_others" else set())
            for name, fns in tabs.items()
        ]
        _br.insert_act_table_loads(self, tables)

    nc.insert_act_table_loads = _act_table_loads_combined_set_only

    # x parts arrive pre-swizzled: xs[p, b, k, n] = x.T[k*128+p, b*CH+n],
    # so each partition's per-pair (2-chunk) DMA slice is contiguous 16KB.
    xh = nc.declare_dram_parameter("xh", [128, nch, KS, CH], F16, isOutput=False)
    xl = nc.declare_dram_parameter("xl", [128, nch, KS, CH], F16, isOutput=False)
    w1h = nc.declare_dram_parameter("w1h", [D, H1], F16, isOutput=False)
    w1l = nc.declare_dram_parameter("w1l", [D, H1], F16, isOutput=False)
    w2t = nc.declare_dram_parameter("w2t", [H1, H2], F16, isOutput=False)
    w5t = nc.declare_dram_parameter("w5t", [H2, O], F16, isOutput=False)
    cs1 = nc.declare_dram_parameter("cs1", [H1, 2], F32, isOutput=False)
    cs2 = nc.declare_dram_parameter("cs2", [H2, 2], F32, isOutput=False)
    b5r = nc.declare_dram_parameter("b5r", [128, 4 * O], F32, isOutput=False)
    eb5r = nc.declare_dram_parameter("eb5r", [128, 4 * O], F32, isOutput=False)
    # Output, transposed blocks: y[j*O + o, b*128 + p] = out[b*512 + j*128 + p, o]
    y = nc.declare_dram_parameter("y", [4 * O, nch * 128], F32, isOutput=True)

    with tile.TileContext(nc) as tc:
        from contextlib import ExitStack

        with ExitStack() as ctx:
            singles = ctx.enter_context(tc.tile_pool(name="singles", bufs=1))
            xpool = ctx.enter_context(tc.tile_pool(name="xpool", bufs=4))
            mids = ctx.enter_context(tc.tile_pool(name="mids", bufs=3))
            outs = ctx.enter_context(tc.tile_pool(name="outs", bufs=3))
            p1pool = ctx.enter_context(tc.tile_pool(name="p1", bufs=2, space="PSUM"))
            p2pool = ctx.enter_context(tc.tile_pool(name="p2", bufs=1, space="PSUM"))
            p5pool = ctx.enter_context(tc.tile_pool(name="p5", bufs=2, space="PSUM"))
            pTpool = ctx.enter_context(tc.tile_pool(name="pT", bufs=1, space="PSUM"))

            # --- constants / weights (loaded once) ---
            w1h_sb = singles.tile([128, KS, H1], F16)
            nc.sync.dma_start(out=w1h_sb, in_=w1h.rearrange("(k p) m -> p k m", p=128))
            w1l_sb = singles.tile([128, KS, H1], F16)
            nc.sync.dma_start(out=w1l_sb, in_=w1l.rearrange("(k p) m -> p k m", p=128))
            w2_sb = singles.tile([H1, H2], F16)
            nc.sync.dma_start(out=w2_sb, in_=w2t[:, :])
            w5_sb = singles.tile([H2, O], F16)
            nc.sync.dma_start(out=w5_sb, in_=w5t[:, :])
            cs1_sb = singles.tile([H1, 2], F32)
            nc.sync.dma_start(out=cs1_sb, in_=cs1[:, :])
            cs2_sb = singles.tile([H2, 2], F32)
            nc.sync.dma_start(out=cs2_sb, in_=cs2[:, :])
            b5r_sb = singles.tile([128, 4 * O], F32)
            nc.sync.dma_start(out=b5r_sb, in_=b5r[:, :])
            eb5r_sb = singles.tile([128, 4 * O], F32)
            nc.sync.dma_start(out=eb5r_sb, in_=eb5r[:, :])
            ident = singles.tile([128, 128], F32)
            make_identity(nc, ident)

            pending = []  # deferred (out_t, chunk) transpose+store

            def store(out_t, b):
                psT = pTpool.tile([4 * O, 128], F32, tag="psT", name="psT")
                nc.tensor.transpose(psT, out_t, ident)
                oT = outs.tile([4 * O, 128], F32, tag="oT", name="oT")
                nc.vector.tensor_copy(oT, psT)
                # SWDGE store: keeps SP's HWDGE FIFO free for input DMAs
                nc.gpsimd.dma_start(out=y[:, b * 128:(b + 1) * 128], in_=oT)

            def stage_a(ps1, b):
                """sign1 -> fc2 -> sign2 for one chunk; returns y2."""
                y1 = mids.tile([H1, CH], F16, tag="y1", name="y1")
                nc.scalar.activation(y1, ps1, AF.Sign,
                                     bias=cs1_sb[:, 1:2], scale=cs1_sb[:, 0:1])
                ps2 = p2pool.tile([H2, CH], F32, tag="ps2", name="ps2")
                nc.tensor.matmul(ps2, lhsT=w2_sb, rhs=y1, start=True, stop=True)
                y2 = mids.tile([H2, CH], F16, tag="y2", name="y2")
                nc.scalar.activation(y2, ps2, AF.Sign,
                                     bias=cs2_sb[:, 1:2], scale=cs2_sb[:, 0:1])
                return y2

            def stage_b(y2, b):
                """fc5 -> log_softmax for one chunk (y2 from a prior pair)."""
                ps5 = p5pool.tile([128, 4, O], F32, tag="ps5", name="ps5")
                for j in range(4):
                    nc.tensor.matmul(ps5[:, j, :], lhsT=y2[:, j * 128:(j + 1) * 128],
                                     rhs=w5_sb, start=True, stop=True)

                # log_softmax along free dim; b5 folded in via exp(b5) weights
                # (|logits| <= 21 so no max-subtraction is needed)
                e = mids.tile([128, 4, O], F32, tag="e", name="e")
                nc.scalar.activation(e, ps5, AF.Exp)
                e2 = mids.tile([128, 4, O], F32, tag="e2", name="e2")
                nc.vector.tensor_tensor(
                    out=e2, in0=e, in1=eb5r_sb.rearrange("p (j o) -> p j o", o=O),
                    op=OP.mult)
                s = mids.tile([128, 4], F32, tag="s", name="s")
                nc.vector.reduce_sum(s, e2, axis=AX.X)
                lse = mids.tile([128, 4], F32, tag="lse", name="lse")
                nc.scalar.activation(lse, s, AF.Ln)
                out_t = outs.tile([128, 4 * O], F32, tag="out", name="out_t")
                for j in range(4):
                    js = slice(j * O, (j + 1) * O)
                    nc.vector.scalar_tensor_tensor(
                        out=out_t[:, js], in0=ps5[:, j, :], scalar=lse[:, j:j + 1],
                        in1=b5r_sb[:, js], op0=OP.subtract, op1=OP.add)
                pending.append((out_t, b))

            # Software pipeline across pairs: fc1(i) streams first; then
            # stage A of pair i-1 (sign1/fc2/sign2 — its ACT inputs completed
            # during fc1(i)), then stage B of pair i-2 (fc5/softmax — its y2
            # completed during fc1(i)), then stores of pairs <= i-3. Every PE
            # instruction's dependencies are already satisfied when the PE's
            # strict FIFO reaches it, so the PE never stalls mid-stream and
            # the HAM clock stays warm; DMA slots recycle at full rate.
            qa, qb = [], []

            def drain(flush=False):
                if qb if flush else (len(qb) > 0 and len(qa) > 0):
                    y2a_, y2b_, b_ = qb.pop(0)
                    stage_b(y2a_, b_)
                    stage_b(y2b_, b_ + 1)
                if qa if flush else (len(qa) > 0):
                    ps1a_, ps1b_, b_ = qa.pop(0)
                    qb.append((stage_a(ps1a_, b_), stage_a(ps1b_, b_ + 1), b_))
                while len(pending) > (2 if not flush else 0):
                    store(*pending.pop(0))

            for pr in range(npair):
                xh_t = xpool.tile([128, 2, KS, CH], F16, tag="xh", name="xh_t")
                nc.sync.dma_start(out=xh_t, in_=xh[:, 2 * pr:2 * pr + 2, :, :])
                xl_t = xpool.tile([128, 2, KS, CH], F16, tag="xl", name="xl_t")
                nc.sync.dma_start(out=xl_t, in_=xl[:, 2 * pr:2 * pr + 2, :, :])

                # fc1 for both chunks of the pair, sharing each LDWEIGHTS
                ps1a = p1pool.tile([H1, CH], F32, tag="ps1a", name="ps1a")
                ps1b = p1pool.tile([H1, CH], F32, tag="ps1b", name="ps1b")
                for k in range(KS):
                    nc.tensor.matmul(ps1a, lhsT=w1h_sb[:, k, :], rhs=xh_t[:, 0, k, :],
                                     start=(k == 0), stop=False)
                    nc.tensor.matmul(ps1b, lhsT=w1h_sb[:, k, :], rhs=xh_t[:, 1, k, :],
                                     start=(k == 0), stop=False)
                    nc.tensor.matmul(ps1a, lhsT=w1l_sb[:, k, :], rhs=xl_t[:, 0, k, :],
                                     start=False, stop=(k == KS - 1))
                    nc.tensor.matmul(ps1b, lhsT=w1l_sb[:, k, :], rhs=xl_t[:, 1, k, :],
                                     start=False, stop=(k == KS - 1))

                drain()
                qa.append((ps1a, ps1b, 2 * pr))

            while qa or qb or pending:
                drain(flush=True)

    nc.finalize()
    return nc


def _prep_inputs(x, w1, b1, g1, be1, m1, v1, w2, b2, g2, be2, m2, v2, w5, b5,
                 bpc: int = BPC, n_cores: int = N_CORES):
    f64 = np.float64
    w1s = np.where(w1 >= 0, 1.0, -1.0).astype(np.float32)
    w2s = np.where(w2 >= 0, 1.0, -1.0).astype(np.float32)
    w5s = np.where(w5 >= 0, 1.0, -1.0).astype(np.float32)

    w1h = np.ascontiguousarray(w1s.T).astype(np.float16)          # [D, H1]
    w1l = (np.ascontiguousarray(w1s.T) / LO).astype(np.float16)   # +-2**-11
    w2t = np.ascontiguousarray(w2s.T).astype(np.float16)          # [H1, H2]
    w5t = np.ascontiguousarray(w5s.T).astype(np.float16)          # [H2, O]

    b5f = b5.astype(np.float32)
    b5r = np.broadcast_to(np.tile(b5f, 4)[None, :], (128, 4 * O)).copy()
    eb5 = np.exp(b5.astype(f64)).astype(np.float32)
    eb5r = np.broadcast_to(np.tile(eb5, 4)[None, :], (128, 4 * O)).copy()

    s1 = (g1.astype(f64) / np.sqrt(v1.astype(f64) + EPS))
    t1 = s1 * (b1.astype(f64) - m1.astype(f64)) + be1.astype(f64)
    cs1 = np.stack([s1, t1], axis=1).astype(np.float32)           # [H1, 2]
    s2 = (g2.astype(f64) / np.sqrt(v2.astype(f64) + EPS))
    t2 = s2 * (b2.astype(f64) - m2.astype(f64)) + be2.astype(f64)
    cs2 = np.stack([s2, t2], axis=1).astype(np.float32)           # [H2, 2]

    x = np.asarray(x, dtype=np.float32)
    xh = x.astype(np.float16)
    xl = ((x - xh.astype(np.float32)) * LO).astype(np.float16)

    def swizzle(a):  # [bpc, D] -> [128, nch, KS, CH] (see build_bass)
        nch = bpc // CH
        return np.ascontiguousarray(
            a.T.reshape(KS, 128, nch, CH).transpose(1, 2, 0, 3))

    in_maps = []
    for c in range(n_cores):
        rs = slice(c * bpc, (c + 1) * bpc)
        in_maps.append({
            "xh": swizzle(xh[rs]),
            "xl": swizzle(xl[rs]),
            "w1h": w1h, "w1l": w1l, "w2t": w2t, "w5t": w5t,
            "cs1": cs1, "cs2": cs2, "b5r": b5r, "eb5r": eb5r,
        })
    return in_maps


def _decode_output(y_dev: np.ndarray, bpc: int) -> np.ndarray:
    # y_dev [4*O, (bpc/512)*128]: y_dev[j*O+o, b*128+p] = out[b*512+j*128+p, o]
    nch = bpc // CH
    return np.ascontiguousarray(
        y_dev.reshape(4, O, nch, 128).transpose(2, 0, 3, 1).reshape(bpc, O))


_CACHED = {}


def kernel(**inputs) -> np.ndarray:
    from concourse.bass_utils import run_bass_kernel_spmd

    in_maps = _prep_inputs(**inputs)
    if "nc" not in _CACHED:
        _CACHED["nc"] = build_bass()
    nc = _CACHED["nc"]
    res = run_bass_kernel_spmd(nc, in_maps, list(range(N_CORES)))
    out = np.empty((B, O), dtype=np.float32)
    for c in range(N_CORES):
        out[c * BPC:(c + 1) * BPC] = _decode_output(res.results[c]["y"], BPC)
    return out
